# revision 3
# baseline (speedup 1.0000x reference)
"""Trainium2 Bass kernel for a NonLocalBlock (B=2, C=256, H=W=64).

Math (reference):
    theta/phi/g = 1x1 conv of inp (C -> CH=128), L = CH*H = 8192
    attn = softmax(th @ ph^T) over kv, with th, ph (L, W=64)
    o = attn @ gg -> out = conv1x1(o) + b_out + inp

Sharding: 8 cores = 2 samples x 4 h-blocks (16 h rows each). Each core
computes the attention output rows q=(ch, h) for its h-block, which is
exactly the data the final conv needs for output columns (h, w) of its
block, so there is no cross-core reduction.

Per-core layouts (kv order = (h', ch') so V tiles come straight from the
g conv output; q order = (h, ch) so attention output transposes directly
into (ch, (h, w)) for the out conv):
    Qt  (64=w,  2048=q)    phT (64=w, 8192=kv)    vaug (128=ch', 64*65)
    S^T tile t = phT[:, t*128:(t+1)*128].T @ Qt   -> psum (128=kv, q)
    E = exp(S^T)  (no max subtraction: logits are within +-80 in fp32)
    O'^T += E.T @ [V_t | 1]  -> psum (65, q): rows 0..63 = o, row 64 = denom
"""

import numpy as np

B, C, H, W = 2, 256, 64, 64
CH = C // 2          # 128
HS = H // 4          # 16 h rows per core
LQ = CH * HS         # 2048 q rows per core
NKV = 64             # kv tiles of 128 (kv = (h', ch'))
QP = 1024            # q per attention pass (PSUM budget)

_cached = {}


def _build_nc():
    import concourse.bass as bass
    import concourse.mybir as mybir
    import concourse.tile as tile
    from concourse import bacc

    f32 = mybir.dt.float32
    AF = mybir.ActivationFunctionType
    ALU = mybir.AluOpType

    nc = bacc.Bacc("TRN2", target_bir_lowering=False, debug=False, num_devices=8)

    x0 = nc.dram_tensor("x0", [128, 4096], f32, kind="ExternalInput")
    x1 = nc.dram_tensor("x1", [128, 4096], f32, kind="ExternalInput")
    xs0 = nc.dram_tensor("xs0", [128, 1024], f32, kind="ExternalInput")
    xs1 = nc.dram_tensor("xs1", [128, 1024], f32, kind="ExternalInput")
    wq = nc.dram_tensor("wq", [128, 2, 128], f32, kind="ExternalInput")
    wp = nc.dram_tensor("wp", [128, 2, 128], f32, kind="ExternalInput")
    wg = nc.dram_tensor("wg", [128, 2, 128], f32, kind="ExternalInput")
    wo = nc.dram_tensor("wo", [128, 2, 128], f32, kind="ExternalInput")
    bth = nc.dram_tensor("bth", [64, 512], f32, kind="ExternalInput")
    bph = nc.dram_tensor("bph", [64, 512], f32, kind="ExternalInput")
    bg = nc.dram_tensor("bg", [128, 1], f32, kind="ExternalInput")
    bo = nc.dram_tensor("bo", [128, 2], f32, kind="ExternalInput")
    ident = nc.dram_tensor("ident", [128, 128], f32, kind="ExternalInput")
    y = nc.dram_tensor("y", [2, 128, 1024], f32, kind="ExternalOutput")

    with tile.TileContext(nc) as tc:
        with (
            tc.tile_pool(name="const", bufs=1) as cp,
            tc.tile_pool(name="big", bufs=1) as bp,
            tc.tile_pool(name="work", bufs=3) as wkp,
        ):
            x0t = bp.tile([128, 4096], f32, tag="x0")
            x1t = bp.tile([128, 4096], f32, tag="x1")
            xs0t = bp.tile([128, 1024], f32, tag="xs0")
            xs1t = bp.tile([128, 1024], f32, tag="xs1")
            nc.sync.dma_start(out=x0t[:], in_=x0[:])
            nc.sync.dma_start(out=x1t[:], in_=x1[:])
            nc.sync.dma_start(out=xs0t[:], in_=xs0[:])
            nc.sync.dma_start(out=xs1t[:], in_=xs1[:])

            wqt = cp.tile([128, 2, 128], f32, tag="wq")
            wpt = cp.tile([128, 2, 128], f32, tag="wp")
            wgt = cp.tile([128, 2, 128], f32, tag="wg")
            wot = cp.tile([128, 2, 128], f32, tag="wo")
            btht = cp.tile([64, 512], f32, tag="bth")
            bpht = cp.tile([64, 512], f32, tag="bph")
            bgt = cp.tile([128, 1], f32, tag="bg")
            bot = cp.tile([128, 2], f32, tag="bo")
            idt = cp.tile([128, 128], f32, tag="ident")
            for dst, src in [(wqt, wq), (wpt, wp), (wgt, wg), (wot, wo),
                             (btht, bth), (bpht, bph), (bgt, bg), (bot, bo),
                             (idt, ident)]:
                nc.sync.dma_start(out=dst[:], in_=src[:])

            qt = bp.tile([64, LQ], f32, tag="qt")          # Qt (w, q)
            pht = bp.tile([64, 8192], f32, tag="pht")      # phT (w, kv)
            vaug = bp.tile([128, NKV * 65], f32, tag="vaug")
            osb = bp.tile([128, 1024], f32, tag="osb")     # o (ch, (h, w))
            otsb = bp.tile([65, LQ], f32, tag="otsb")      # O'^T staged in SBUF

            # ---- phase 1: Qt and phT straight from x via per-h matmuls ----
            with tc.tile_pool(name="pA", bufs=3, space="PSUM") as pA:
                # phi for all 64 h' of the sample, batches of 4 h per bank
                for g4 in range(16):
                    ps = pA.tile([64, 512], f32, tag="a")
                    for hh in range(4):
                        h = g4 * 4 + hh
                        nc.tensor.matmul(ps[:, hh * 128:(hh + 1) * 128],
                                         lhsT=x0t[:, h * 64:(h + 1) * 64],
                                         rhs=wpt[:, 0, :], start=True, stop=False)
                        nc.tensor.matmul(ps[:, hh * 128:(hh + 1) * 128],
                                         lhsT=x1t[:, h * 64:(h + 1) * 64],
                                         rhs=wpt[:, 1, :], start=False, stop=True)
                    nc.vector.tensor_tensor(out=pht[:, g4 * 512:(g4 + 1) * 512],
                                            in0=ps[:], in1=bpht[:], op=ALU.add)
                # theta for this core's 16 h rows (local columns of xs)
                for g4 in range(4):
                    ps = pA.tile([64, 512], f32, tag="a")
                    for hh in range(4):
                        lh = g4 * 4 + hh
                        nc.tensor.matmul(ps[:, hh * 128:(hh + 1) * 128],
                                         lhsT=xs0t[:, lh * 64:(lh + 1) * 64],
                                         rhs=wqt[:, 0, :], start=True, stop=False)
                        nc.tensor.matmul(ps[:, hh * 128:(hh + 1) * 128],
                                         lhsT=xs1t[:, lh * 64:(lh + 1) * 64],
                                         rhs=wqt[:, 1, :], start=False, stop=True)
                    nc.vector.tensor_tensor(out=qt[:, g4 * 512:(g4 + 1) * 512],
                                            in0=ps[:], in1=btht[:], op=ALU.add)

                # ---- phase 2: g conv -> vaug (with ones column per tile) ----
                vaug3 = vaug.rearrange("p (t j) -> p t j", j=65)
                for n in range(8):
                    ps = pA.tile([128, 512], f32, tag="a")
                    nc.tensor.matmul(ps[:], lhsT=wgt[:, 0, :],
                                     rhs=x0t[:, n * 512:(n + 1) * 512],
                                     start=True, stop=False)
                    nc.tensor.matmul(ps[:], lhsT=wgt[:, 1, :],
                                     rhs=x1t[:, n * 512:(n + 1) * 512],
                                     start=False, stop=True)
                    nc.scalar.add(vaug3[:, n * 8:(n + 1) * 8, 0:64],
                                  ps.rearrange("p (t j) -> p t j", j=64)[:],
                                  bgt[:, 0:1])
                nc.vector.memset(vaug3[:, :, 64:65], 1.0)

            # ---- phase 3: attention, two q passes of 1024 ----
            with (
                tc.tile_pool(name="pS", bufs=2, space="PSUM") as pS,
                tc.tile_pool(name="pOT", bufs=1, space="PSUM") as pOT,
            ):
                for p in range(2):
                    otp = pOT.tile([65, QP], f32, tag="ot")
                    for t in range(NKV):
                        sp = pS.tile([128, QP], f32, tag="s")
                        for c in range(2):
                            nc.tensor.matmul(
                                sp[:, c * 512:(c + 1) * 512],
                                lhsT=pht[:, t * 128:(t + 1) * 128],
                                rhs=qt[:, p * QP + c * 512: p * QP + (c + 1) * 512],
                                start=True, stop=True)
                        et = wkp.tile([128, QP], f32, tag="e")
                        nc.scalar.activation(et[:], sp[:], AF.Exp)
                        for c in range(2):
                            nc.tensor.matmul(
                                otp[:, c * 512:(c + 1) * 512],
                                lhsT=vaug3[:, t, :],
                                rhs=et[:, c * 512:(c + 1) * 512],
                                start=(t == 0), stop=(t == NKV - 1),
                                skip_group_check=True)
                    nc.vector.tensor_copy(otsb[:, p * QP:(p + 1) * QP], otp[:])

            # ---- phase 4: transpose + normalize -> osb (ch, (h, w)) ----
            with (
                tc.tile_pool(name="pTR", bufs=2, space="PSUM") as pTR,
                tc.tile_pool(name="pY", bufs=2, space="PSUM") as pY,
            ):
                for lh in range(16):
                    trp = pTR.tile([128, 65], f32, tag="tr")
                    nc.tensor.transpose(trp[:], otsb[:, lh * 128:(lh + 1) * 128],
                                        idt[0:65, 0:65])
                    rden = wkp.tile([128, 1], f32, tag="rden")
                    nc.vector.reciprocal(rden[:], trp[:, 64:65])
                    nc.vector.tensor_scalar(
                        out=osb[:, lh * 64:(lh + 1) * 64], in0=trp[:, 0:64],
                        scalar1=rden[:], scalar2=None, op0=ALU.mult)

                # ---- phase 5: out conv + bias + residual ----
                for m in range(2):
                    ysb = wkp.tile([128, 1024], f32, tag="y")
                    xres = xs0t if m == 0 else xs1t
                    for n2 in range(2):
                        yp = pY.tile([128, 512], f32, tag="yp")
                        nc.tensor.matmul(yp[:], lhsT=wot[:, m, :],
                                         rhs=osb[:, n2 * 512:(n2 + 1) * 512],
                                         start=True, stop=True)
                        nc.vector.tensor_scalar(
                            out=ysb[:, n2 * 512:(n2 + 1) * 512], in0=yp[:],
                            scalar1=bot[:, m:m + 1], scalar2=None, op0=ALU.add)
                    nc.vector.tensor_tensor(out=ysb[:], in0=ysb[:], in1=xres[:],
                                            op=ALU.add)
                    nc.sync.dma_start(out=y[m], in_=ysb[:])

    nc.compile()
    return nc


def _get_nc():
    if "nc" not in _cached:
        _cached["nc"] = _build_nc()
    return _cached["nc"]


LAST_EXEC_NS = None
LAST_TRACE_DIR = None


def kernel(inp, w_theta, b_theta, w_phi, b_phi, w_g, b_g, w_out, b_out):
    import os
    from concourse.bass_utils import run_bass_kernel_spmd

    nc = _get_nc()

    f = np.float32
    c = np.ascontiguousarray

    def wT3(w):  # (CH, C) -> (128, 2, 128): [c_in_lo, half, ch]
        return c(w.T.reshape(2, 128, CH).transpose(1, 0, 2).astype(f))

    wq3, wp3, wg3 = wT3(w_theta), wT3(w_phi), wT3(w_g)
    wo3 = c(w_out.reshape(2, 128, CH).transpose(2, 0, 1).astype(f))  # [ch, m, co]
    bth4 = c(np.tile(b_theta.astype(f), (64, 4)))
    bph4 = c(np.tile(b_phi.astype(f), (64, 4)))
    bg1 = c(b_g.astype(f)[:, None])
    bo2 = c(b_out.reshape(2, 128).T.astype(f))
    ident = np.eye(128, dtype=f)

    in_maps = []
    for core in range(8):
        b, k = core // 4, core % 4
        x = inp[b].reshape(C, H * W).astype(f)
        in_maps.append({
            "x0": c(x[:128]), "x1": c(x[128:]),
            "xs0": c(x[:128, 1024 * k:1024 * (k + 1)]),
            "xs1": c(x[128:, 1024 * k:1024 * (k + 1)]),
            "wq": wq3, "wp": wp3, "wg": wg3, "wo": wo3,
            "bth": bth4, "bph": bph4, "bg": bg1, "bo": bo2, "ident": ident,
        })

    trace = bool(os.environ.get("NLB_TRACE"))
    tmpdir = os.environ.get("NLB_TRACE_DIR") or None
    res = run_bass_kernel_spmd(nc, in_maps, list(range(8)), trace=trace,
                               tmpdir=tmpdir)
    global LAST_EXEC_NS, LAST_TRACE_DIR
    LAST_EXEC_NS = res.exec_time_ns
    LAST_TRACE_DIR = tmpdir

    out = np.empty((B, C, H, W), dtype=f)
    for core in range(8):
        b, k = core // 4, core % 4
        yc = res.results[core]["y"].reshape(C, HS, W)
        out[b, :, HS * k:HS * (k + 1), :] = yc
    return out
